# revision 1
# baseline (speedup 1.0000x reference)
"""Trainium2 Bass kernel for nn_CoordinateConditioning.

out[i,j,h] = v[i,j]*( (X[i]-X[j])@Wcoord[h] + Wdist[h]*R[i,j] + B*Wmask[h] )
             + C[i,h] + C[j,h]
with X = sum_b coords[b], R[i,j] = sum_b 1/(1+||x_b[i]-x_b[j]||^2),
v = pad/uid mask, C = B*c0 + gathered s_to_c sum.

Key structure exploited: ref_space_uid is sorted, so v[i,j] is a narrow
block-diagonal band.  For each core (256 i-rows) only W (~3) of the 16
j-tiles can contain v!=0 pairs; host computes the per-core window-tile
list from the actual inputs and the device program processes exactly
W "window" tiles (full geometry pipeline) + 16-W "dense" tiles
(out = C_i + C_j only, K=17 matmul with a shared stationary operand).
Per-core j-tile order is a host-chosen permutation sigma_c; the host
inverse-permutes the j-tile blocks when assembling the full output.

Output is written fp16 (rel tolerance is 2e-2; fp16 rounding ~5e-4),
halving the dominant out-DMA traffic.  PSUM->SBUF copies are split
between ACT and DVE; SBUF-only elementwise runs on GPSIMD.
"""

import numpy as np
from contextlib import ExitStack

B, N, T, TOKEN_S, DIM_F, H = 4, 2048, 256, 384, 256, 16
NCORES = 8
IB = N // NCORES          # 256 i rows per core
NJT = N // 128            # 16 j tiles
KB = 17                   # K rows per batch for the r2 matmul
KU = 2                    # uid delta rows
KD = 4                    # rows per D_k
KF = 4 * KB + KU + 3 * KD # 82 total J/I feature rows
BIGM = 4096.0

_CACHE = {}


def _split_hi_lo(v):
    """fp16-exact hi/lo split (hi keeps 10 mantissa bits)."""
    v = np.ascontiguousarray(v, dtype=np.float32)
    hi = (v.view(np.uint32) & np.uint32(0xFFFFE000)).view(np.float32)
    return hi, (v - hi).astype(np.float32)


def _host_tables(inputs):
    I = {k: np.asarray(v) for k, v in inputs.items()}
    x = np.ascontiguousarray(I['atom_coords_noisy'], dtype=np.float32)  # [B,N,3]
    m = I['atom_pad_mask'].reshape(-1).astype(np.float32)               # [N]
    uid = I['ref_space_uid'].reshape(-1).astype(np.float32)             # [N]

    # ---- small linears (replicated) ----
    def ln(v, g, b, eps=1e-5):
        mu = v.mean(-1, keepdims=True)
        var = ((v - mu) ** 2).mean(-1, keepdims=True)
        return (v - mu) / np.sqrt(var + eps) * g + b

    s = np.concatenate([I['s_trunk'], I['s_inputs']], -1).astype(np.float32) @ I['W_single'].T
    fe = np.cos(2 * np.pi * (I['times'][:, None] * I['Wf'][:, 0][None, :] + I['bf'])).astype(np.float32)
    s = s + (ln(fe, I['ln_f_g'], I['ln_f_b']) @ I['Wf2s'].T)[:, None, :]
    s2c = ln(s, I['ln_s_g'], I['ln_s_b']) @ I['Wsc'].T                  # [B,T,1]
    ssum = s2c[:, :, 0].sum(0)                                          # [T]
    tok = I['atom_to_token_idx'].reshape(-1).astype(np.int64)
    S = ssum[tok]                                                       # [N]
    af = np.concatenate([I['ref_pos'][0], I['ref_charge'][0][:, None],
                         I['ref_element'][0]], -1).astype(np.float32)   # [N,132]
    c0 = af @ I['Wa'].T + I['ba']                                       # [N,16]
    C = (B * c0 + S[:, None]).astype(np.float32)                        # [N,16]

    X = x.sum(0)                                                        # [N,3]
    Wc = np.asarray(I['Wcoord'], np.float32)                            # [16,3]
    # device feature maps hold X_j - X_i, the formula needs X_i - X_j -> negate
    wtab = np.stack([-Wc[:, 0], -Wc[:, 1], -Wc[:, 2],
                     np.asarray(I['Wdist'], np.float32)[:, 0],
                     B * np.asarray(I['Wmask'], np.float32)[:, 0]], 0)  # [5,16]

    # ---- J/I feature tables for the per-(j,i) matmuls ----
    n2 = np.einsum('bnk,bnk->bn', x.astype(np.float64), x.astype(np.float64)).astype(np.float32)
    ones = np.ones(N, np.float32)
    jf = np.zeros((KF, N), np.float32)
    itab = np.zeros((KF, N), np.float32)
    for b in range(B):
        r = b * KB
        for k in range(3):
            xh, xl = _split_hi_lo(x[b, :, k])
            jf[r + 4 * k + 0] = xh
            jf[r + 4 * k + 1] = xh
            jf[r + 4 * k + 2] = xl
            jf[r + 4 * k + 3] = xl
            itab[r + 4 * k + 0] = -2.0 * xh
            itab[r + 4 * k + 1] = -2.0 * xl
            itab[r + 4 * k + 2] = -2.0 * xh
            itab[r + 4 * k + 3] = -2.0 * xl
        nh, nl = _split_hi_lo(n2[b])
        jf[r + 12], jf[r + 13] = nh, nl
        itab[r + 12], itab[r + 13] = ones, ones
        jf[r + 14], jf[r + 15] = ones, ones
        itab[r + 14], itab[r + 15] = nh, nl
        jf[r + 16] = ones
        itab[r + 16] = ones
    ru = 4 * KB
    jf[ru] = uid + BIGM * (1.0 - m)
    itab[ru] = ones
    jf[ru + 1] = ones
    itab[ru + 1] = -uid + BIGM * (1.0 - m)
    for k in range(3):
        r = ru + KU + KD * k
        Xh, Xl = _split_hi_lo(X[:, k])
        jf[r + 0], jf[r + 1] = Xh, Xl
        itab[r + 0], itab[r + 1] = ones, ones
        jf[r + 2], jf[r + 3] = ones, ones
        itab[r + 2], itab[r + 3] = -Xh, -Xl

    # ---- constant rhs pattern rows (per 256-col jsub block) ----
    # rows 0..79: geometry block-diag(delta_jp * wtab[t]); 80..95: delta_h
    blk = np.zeros((96, 256), np.float32)
    for t in range(5):
        for jp in range(16):
            blk[t * 16 + jp, jp * 16:(jp + 1) * 16] = wtab[t]
    for hp in range(16):
        blk[80 + hp, hp::16] = 1.0
    pc = np.tile(blk, (1, 8))                                           # [96, 2048]

    # bake the per-tile column permutation p -> j = (p%8)*16 + p//8 into jf
    # so device lhsT slices are plain contiguous (walrus: one free dim only)
    p = np.arange(128)
    perm = (np.arange(N) // 128) * 128 + ((p % 8) * 16 + p // 8)[np.tile(p, N // 128) * 0 + np.arange(N) % 128]
    jf = np.ascontiguousarray(jf[:, perm])

    cflat = C.reshape(1, N * H).astype(np.float32)
    return jf, itab, pc, C, cflat, m.astype(bool), uid


def _windows(m, uid):
    """Per-core window j-tile lists (tiles that can hold v!=0 pairs) and
    the per-core slot->j-tile permutation sigma (window tiles first)."""
    tiles_per_core = []
    for c in range(NCORES):
        sl = slice(c * IB, (c + 1) * IB)
        vi = m[sl]
        if vi.any():
            U = np.unique(uid[sl][vi])
            pj = np.where(m & np.isin(uid, U))[0]
            tiles = sorted(set((pj // 128).tolist()))
        else:
            tiles = []
        tiles_per_core.append(tiles)
    W = max(1, max(len(t) for t in tiles_per_core))
    sigmas = []
    for tiles in tiles_per_core:
        rest = [t for t in range(NJT) if t not in tiles]
        pad = rest[:W - len(tiles)]
        rest2 = rest[W - len(tiles):]
        sigmas.append(np.array(tiles + pad + rest2, np.int64))
    return sigmas, W


def _build_program(W):
    key = ('nc', W)
    if key in _CACHE:
        return _CACHE[key]
    import concourse.bass as bass
    import concourse.bacc as bacc
    import concourse.tile as tile
    from concourse import mybir

    f32 = mybir.dt.float32
    f32r = mybir.dt.float32r
    f16 = mybir.dt.float16

    nc = bacc.Bacc("TRN2", target_bir_lowering=False, debug=False)
    j4 = nc.dram_tensor("j4", [KB, B * W * 128], f16, kind="ExternalInput").ap()
    i4 = nc.dram_tensor("i4", [KB, B * IB], f16, kind="ExternalInput").ap()
    j3 = nc.dram_tensor("j3", [KD, 3 * W * 128], f16, kind="ExternalInput").ap()
    i3 = nc.dram_tensor("i3", [KD, 3 * IB], f16, kind="ExternalInput").ap()
    ju = nc.dram_tensor("ju", [KU, W * 128], f32r, kind="ExternalInput").ap()
    iu = nc.dram_tensor("iu", [KU, IB], f32r, kind="ExternalInput").ap()
    pbd = nc.dram_tensor("pbd", [17, NJT * 2048], f16, kind="ExternalInput").ap()
    pbw = nc.dram_tensor("pbw", [97, W * 2048], f16, kind="ExternalInput").ap()
    lwc = nc.dram_tensor("lwc", [17, 2048], f16, kind="ExternalInput").ap()
    ldc = nc.dram_tensor("ldc", [17, IB], f16, kind="ExternalInput").ap()
    outp = nc.dram_tensor("outp", [IB, N * H], f16, kind="ExternalOutput").ap()



    with tile.TileContext(nc) as tc:
        with ExitStack() as ctx:
            cpool = ctx.enter_context(tc.tile_pool(name="const", bufs=1))
            # matmul operands need base partition 0; same-K groups share one
            # tile, sliced along the free dim (6 loads instead of 18)
            J4 = cpool.tile([KB, B * W * 128], f16, tag="J4")
            I4 = cpool.tile([KB, B * IB], f16, tag="I4")
            J3 = cpool.tile([KD, 3 * W * 128], f16, tag="J3")
            I3 = cpool.tile([KD, 3 * IB], f16, tag="I3")
            Ju = cpool.tile([KU, W * 128], f32r, tag="Ju")
            Iu = cpool.tile([KU, IB], f32r, tag="Iu")
            nc.scalar.dma_start(J4[:, :], j4[:, :])
            nc.scalar.dma_start(I4[:, :], i4[:, :])
            nc.scalar.dma_start(Ju[:, :], ju[:, :])
            nc.scalar.dma_start(Iu[:, :], iu[:, :])
            nc.scalar.dma_start(J3[:, :], j3[:, :])
            nc.scalar.dma_start(I3[:, :], i3[:, :])
            # rhs patterns: row 0 = Cj flat (per step), window rows 1..81 geo,
            # 81..97 delta_h; dense rows 1..17 delta_h
            # static rhs, one load each: rows 0 = Cj flat per slot, 1..17 =
            # delta_h, (window) 17..97 = geometry pattern
            PBD = cpool.tile([17, NJT * 2048], f16, tag="PBD")
            PBW = cpool.tile([97, W * 2048], f16, tag="PBW")
            nc.sync.dma_start(PBD[:, :], pbd[:, :])
            nc.scalar.dma_start(PBW[:, :], pbw[:, :])
            # dense lhsT: row 0 = ones, rows 1..17 = Ci^T  (shared by all dense MMs)
            Ld = cpool.tile([17, IB], f16, tag="Ld")
            nc.sync.dma_start(Ld[:, :], ldc[:, :])
            # window lhsT per slot: rows 0..17 = [ones; Ci], 17..97 geometry (repack)
            Lw = []
            for s in range(W):
                Lt = cpool.tile([97, 2048], f16, tag=f"Lw{s}")
                nc.scalar.dma_start(Lt[0:17, :], lwc[:, :])
                Lw.append(Lt)

            psS = ctx.enter_context(tc.tile_pool(name="psS", bufs=1, space="PSUM"))
            psO = ctx.enter_context(tc.tile_pool(name="psO", bufs=3, space="PSUM"))
            wk = ctx.enter_context(tc.tile_pool(name="wk", bufs=2))
            stg = ctx.enter_context(tc.tile_pool(name="stg", bufs=3))

            # ---- stage 1: geometry features for the W window slots ----
            for s in range(W):
                ps1 = psS.tile([128, 1024], f32, tag="s1", name="ps1")
                for b in range(B):
                    nc.tensor.matmul(ps1[:, b * 256:(b + 1) * 256],
                                     J4[:, (b * W + s) * 128:(b * W + s + 1) * 128],
                                     I4[:, b * IB:(b + 1) * IB],
                                     start=True, stop=True)
                rc = wk.tile([128, 1024], f32, tag="rc")
                nc.vector.reciprocal_approx_fast(rc[:, :], ps1[:, :])
                # m reuses the same stage-1 PSUM bank after recip drained r2
                ps2 = psS.tile([128, 1024], f32, tag="s1", name="ps2")
                nc.tensor.matmul(ps2[:, 0:256],
                                 Ju[:, s * 128:(s + 1) * 128], Iu[:, :],
                                 start=True, stop=True)
                for k in range(3):
                    nc.tensor.matmul(ps2[:, 256 + k * 256:512 + k * 256],
                                     J3[:, (k * W + s) * 128:(k * W + s + 1) * 128],
                                     I3[:, k * IB:(k + 1) * IB],
                                     start=True, stop=True)
                r2h = wk.tile([128, 512], f32, tag="r2h")
                Rt = wk.tile([128, 256], f32, tag="Rt")
                nc.gpsimd.tensor_add(r2h[:, :], rc[:, 0:512], rc[:, 512:1024])
                nc.gpsimd.tensor_add(Rt[:, :], r2h[:, 0:256], r2h[:, 256:512])

                F5 = wk.tile([128, 1280], f16, tag="F5")
                vt = wk.tile([128, 256], f32, tag="vt")
                nc.vector.tensor_scalar(vt[:, :], ps2[:, 0:256], 0.0, None,
                                        op0=mybir.AluOpType.is_equal)
                for k in range(3):
                    nc.vector.tensor_mul(F5[:, k * 256:(k + 1) * 256],
                                         vt[:, :], ps2[:, 256 + k * 256:512 + k * 256])
                nc.vector.tensor_mul(F5[:, 768:1024], vt[:, :], Rt[:, :])
                nc.gpsimd.tensor_copy(F5[:, 1024:1280], vt[:, :])

                # repack: permuted partitions make each feature's src contiguous
                for t in range(5):
                    dst = Lw[s][17 + t * 16:17 + (t + 1) * 16, :].rearrange(
                        "k (a i) -> k a i", a=8)
                    nc.gpsimd.dma_start(dst, F5[:, t * 256:(t + 1) * 256])

            # ---- output steps: dense warmup, windows mid-stream, dense tail ----
            dense_list = list(range(W, NJT))
            lead = min(8, len(dense_list))
            slots_order = dense_list[:lead]
            tail = dense_list[lead:]
            for w in range(W):
                slots_order.append(w)
                if w < len(tail):
                    slots_order.append(tail[w])
            slots_order += tail[W:]
            for k, slot in enumerate(slots_order):
                dense = slot >= W
                a = 4 if k < 4 else 2  # ACT share of 4 copies
                qidx = 0
                st = stg.tile([128, 4096], f16, tag="st")
                for it in range(2):
                    for g in range(2):
                        po = psO.tile([128, 1024], f32, tag="po")
                        if dense:
                            for jl in range(2):
                                c0 = slot * 2048 + g * 1024 + jl * 512
                                nc.tensor.matmul(
                                    po[:, jl * 512:(jl + 1) * 512],
                                    Ld[0:17, it * 128:(it + 1) * 128],
                                    PBD[0:17, c0:c0 + 512],
                                    start=True, stop=True)
                        else:
                            for jl in range(4):
                                js = g * 4 + jl
                                base = js * 256 + it * 128
                                nc.tensor.matmul(
                                    po[:, jl * 256:(jl + 1) * 256],
                                    Lw[slot][0:97, base:base + 128],
                                    PBW[0:97, slot * 2048 + js * 256:slot * 2048 + (js + 1) * 256],
                                    start=True, stop=True)
                        dst = st[:, it * 2048 + g * 1024:it * 2048 + (g + 1) * 1024]
                        if qidx < a:
                            nc.scalar.copy(dst, po[:, :])
                        else:
                            nc.vector.tensor_copy(dst, po[:, :])
                        qidx += 1
                dstv = outp.rearrange("(t p) nh -> p t nh", t=2)
                nc.sync.dma_start(
                    dstv[:, :, slot * 2048:(slot + 1) * 2048],
                    st[:, :].rearrange("p (t c) -> p t c", t=2))
    nc.compile()
    _CACHE[key] = nc
    return nc


def make_in_maps(inputs):
    jf, itab, pc, C, cflat, m, uid = _host_tables(inputs)
    sigmas, W = _windows(m, uid)
    pcw16 = pc.astype(np.float16)
    pcd16 = np.ascontiguousarray(pc[80:96]).astype(np.float16)
    in_maps = []
    ru = 4 * KB
    for c in range(NCORES):
        sl = slice(c * IB, (c + 1) * IB)
        sg = sigmas[c]
        jfw = np.concatenate([jf[:, t * 128:(t + 1) * 128] for t in sg[:W]], axis=1)
        cflP = np.concatenate([cflat[:, t * 2048:(t + 1) * 2048] for t in sg], axis=1)
        citc = np.ascontiguousarray(C.T[:, sl]).astype(np.float16)   # [16, IB]
        cf16 = np.ascontiguousarray(cflP).astype(np.float16)
        pbd_all = np.empty((17, NJT * 2048), np.float16)
        pbd_all[0] = cf16[0]
        pbd_all[1:17] = np.tile(pcd16, (1, NJT))
        pbw_all = np.empty((97, W * 2048), np.float16)
        pbw_all[0] = cf16[0, :W * 2048]
        pbw_all[1:17] = np.tile(pcd16, (1, W))
        pbw_all[17:97] = np.tile(pcw16[0:80], (1, W))
        ldc = np.concatenate([np.ones((1, IB), np.float16), citc], 0)
        lwc = np.concatenate([np.ones((1, 2048), np.float16),
                              np.tile(citc, (1, 8))], 0)
        ifc = itab[:, sl]
        in_maps.append({
            "j4": np.concatenate([jfw[b * KB:(b + 1) * KB] for b in range(B)],
                                 axis=1).astype(np.float16),
            "i4": np.concatenate([ifc[b * KB:(b + 1) * KB] for b in range(B)],
                                 axis=1).astype(np.float16),
            "j3": np.concatenate([jfw[ru + KU + KD * k:ru + KU + KD * (k + 1)]
                                  for k in range(3)], axis=1).astype(np.float16),
            "i3": np.concatenate([ifc[ru + KU + KD * k:ru + KU + KD * (k + 1)]
                                  for k in range(3)], axis=1).astype(np.float16),
            "ju": np.ascontiguousarray(jfw[ru:ru + KU]),
            "iu": np.ascontiguousarray(ifc[ru:ru + KU]),
            "pbd": pbd_all,
            "pbw": pbw_all,
            "lwc": np.ascontiguousarray(lwc),
            "ldc": np.ascontiguousarray(ldc),
        })
    return in_maps, sigmas, W


def _assemble(res, sigmas):
    out = np.empty((1, N, N, H), np.float32)
    for c in range(NCORES):
        dev = np.asarray(res.results[c]["outp"]).astype(np.float32)
        dev = dev.reshape(IB, NJT, 128 * H)
        blk = out[0, c * IB:(c + 1) * IB].reshape(IB, NJT, 128 * H)
        blk[:, sigmas[c], :] = dev
    return out


def kernel(**inputs):
    from concourse import bass_utils
    in_maps, sigmas, W = make_in_maps(inputs)
    nc = _build_program(W)
    res = bass_utils.run_bass_kernel_spmd(nc, in_maps, core_ids=list(range(NCORES)))
    return _assemble(res, sigmas)

